# revision 2
# baseline (speedup 1.0000x reference)
"""Trainium2 Bass kernel for GQA causal attention (nn_Attention_89816356094768).

Math (per reference):
  q = x @ wq.T + bq ; k = x @ wk.T + bk ; v = x @ wv.T + bv
  RoPE on q, k; S = q @ k.T * D**-0.5 with causal mask; P = softmax(S)
  out = (P @ v) reassembled over heads @ wo.T

Sharding: tensor-parallel over heads across 8 cores. Core c owns q heads
(2c, 2c+1) and kv head c//4. Each core computes its two heads' attention and
a row-parallel partial of the output projection; the host sums the 8 partials
(fp16 partials, summed in fp32).

v2 changes vs the bf16 baseline:
- q/k/v and output projections run as fp8e4m3 DoubleRow matmuls (contraction
  pairs packed on the partition dim, 0.5 cycles/row on half-width output =
  4x bf16 per k-tile). Full precision is kept with a hi+lo split of both
  operands and 3 cross terms (hi*hi + hi*lo + lo*hi), costing 0.75x the bf16
  cycles. Weights are pre-scaled by 64 on the host (w std 0.012 sits below
  e4m3's normal range; the 1/64 is folded into downstream scalars); the
  attention output is scaled by 8 via the rowsum ones-matrix (host-side
  ones/8) before its fp8 split, and the o-projection PSUM is divided by 512
  in the PSUM->fp16 copy.
- RoPE's rotate-half matmul is gone: both rope products are computed straight
  from the projection PSUM with scalar_tensor_tensor ((pl + 64b) * table/64,
  bias fused for free), and the sin-product is partition-rotated with a
  2-descriptor SBUF->SBUF DMA using a host-rotated signed sin table.
- Attention (scores/softmax/PV/rowsum) stays bf16: scores contract over only
  d=128 so DoubleRow cannot beat bf16 there, and P would need an on-device
  hi/lo split (2 extra full passes over the score matrix) to keep precision.
- Output partials are fp16 (half the store traffic), one store per t-tile.
Attention units are emitted under tc.high_priority(offset=110) so projection
work drifts into the exp-latency stall windows as PE filler.
"""

import numpy as np
import ml_dtypes
from contextlib import ExitStack

from concourse import bacc, tile, mybir
from concourse.bass_utils import run_bass_kernel_spmd

NQ, NKV, D = 16, 2, 128
HID = 2048
T = 4096
SCALE = D ** -0.5
NCORES = 8
HPC = NQ // NCORES          # q heads per core
P = 128                     # partitions
TS = 512                    # t-slice width (matmul moving free dim)
NT = T // P                 # 32 t tiles
NSL = T // TS               # 8 t slices
HO = HID // P               # 16 hidden k-tiles
BF16 = mybir.dt.bfloat16
F32 = mybir.dt.float32
F16 = mybir.dt.float16
F8 = mybir.dt.float8e4
AF = mybir.ActivationFunctionType
ALU = mybir.AluOpType
DR = mybir.MatmulPerfMode.DoubleRow
NPBF16 = ml_dtypes.bfloat16
NPF8 = ml_dtypes.float8_e4m3

WSCL = 64.0                 # fp8 pre-scale on weights
AOSCL = 8.0                 # fp8 pre-scale on attention output (via ones/8)

_CACHE = {}


def _emit(nc, io, o_dram):
    with ExitStack() as top:
        tc = top.enter_context(tile.TileContext(nc))
        const = top.enter_context(tc.tile_pool(name="const", bufs=1))
        persist = top.enter_context(tc.tile_pool(name="persist", bufs=1))

        def cload(name, shape, dt, eng=None):
            t = const.tile(shape, dt, tag=name)
            (eng or nc.sync).dma_start(t[:], io[name][:])
            return t

        xs_pool = top.enter_context(tc.tile_pool(name="xs", bufs=2))

        # Load order: the first projection term (w_hi * x_hi) only needs the
        # hi parts, so stream those first in chunks, then the lo parts, then
        # phase-B constants.
        wqh = const.tile([P, HO, HPC * D], F8, tag="wqth")
        wql = const.tile([P, HO, HPC * D], F8, tag="wqtl")
        xt0h = xs_pool.tile([P, HO, TS], F8, tag="xth", name="xt0h")
        xt0l = xs_pool.tile([P, HO, TS], F8, tag="xtl", name="xt0l")
        bq = wkh = wvh = wkl = wvl = None
        for ch in range(8):
            hs = slice(2 * ch, 2 * (ch + 1))
            nc.sync.dma_start(wqh[:, hs, :], io["wqth"][:, hs, :])
            nc.sync.dma_start(xt0h[:, hs, :], io["xtth"][0, :, hs, :])
            if ch == 1:
                bq = cload("bq", [P, HPC], F32)
            elif ch == 3:
                wkh = cload("wkth", [P, HO, D], F8)
            elif ch == 5:
                wvh = cload("wvth", [P, HO, D], F8)
            elif ch == 6:
                nc.sync.dma_start(xt0l[:, :8, :], io["xttl"][0, :, :8, :])
            elif ch == 7:
                nc.sync.dma_start(xt0l[:, 8:, :], io["xttl"][0, :, 8:, :])
        bk = cload("bk", [P, 1], F32)
        bv = cload("bv", [P, 1], F32)
        wql_l = cload("wqtl", [P, HO, HPC * D], F8)
        wkl = cload("wktl", [P, HO, D], F8)
        wvl = cload("wvtl", [P, HO, D], F8)
        wql = wql_l
        iden = cload("iden", [P, P], BF16)
        xt1h = xs_pool.tile([P, HO, TS], F8, tag="xth", name="xt1h")
        xt1l = xs_pool.tile([P, HO, TS], F8, tag="xtl", name="xt1l")
        nc.sync.dma_start(xt1h[:], io["xtth"][1])
        nc.sync.dma_start(xt1l[:], io["xttl"][1])
        # RoPE tables stream per t-slice inside phase A
        cosT = const.tile([P, T], BF16, tag="cost")    # cos/64
        sinTr = const.tile([P, T], BF16, tag="sintr")  # signed, rotated, /64
        ones = cload("ones", [P, P], BF16, eng=nc.gpsimd)   # value 1/8
        tri = cload("tri", [P, P], BF16, eng=nc.gpsimd)
        woh = const.tile([P, HPC, HID], F8, tag="woth")  # loaded at sl==2
        wol = const.tile([P, HPC, HID], F8, tag="wotl")

        qT = persist.tile([P, HPC, T], BF16, tag="qT")     # [d, h, t]
        kT = persist.tile([P, T], BF16, tag="kT")          # [d, s]
        vN = persist.tile([P, NT, P], BF16, tag="vN")      # [s_in, s_tile, d]
        aoh = persist.tile([P, HPC, T], F8, tag="aoh")     # [d, h, t] hi
        aol = persist.tile([P, HPC, T], F8, tag="aol")     # [d, h, t] lo

        # ---- Phase A: q/k/v fp8-DR projections, RoPE, v transpose ----
        with ExitStack() as pa:
            ppsum = pa.enter_context(tc.tile_pool(name="ppsum", bufs=5, space="PSUM"))
            vpsum = pa.enter_context(tc.tile_pool(name="vpsum", bufs=2, space="PSUM"))
            rtmp = pa.enter_context(tc.tile_pool(name="rtmp", bufs=4))

            for sl in range(NSL):
                tsl = slice(sl * TS, (sl + 1) * TS)
                if sl == 0:
                    xth, xtl = xt0h, xt0l
                elif sl == 1:
                    xth, xtl = xt1h, xt1l
                else:
                    xth = xs_pool.tile([P, HO, TS], F8, tag="xth")
                    xtl = xs_pool.tile([P, HO, TS], F8, tag="xtl")
                    nc.sync.dma_start(xth[:], io["xtth"][sl])
                    nc.sync.dma_start(xtl[:], io["xttl"][sl])
                nc.sync.dma_start(cosT[:, tsl], io["cost"][:, tsl])
                nc.sync.dma_start(sinTr[:, tsl], io["sintr"][:, tsl])
                if sl == 2:
                    nc.sync.dma_start(woh[:], io["woth"][:])
                elif sl == 3:
                    nc.sync.dma_start(wol[:], io["wotl"][:])

                # (hi weight AP, lo weight AP, bias, kind, head idx)
                jobs = [(wqh[:, :, h * D:(h + 1) * D],
                         wql[:, :, h * D:(h + 1) * D],
                         bq[:, h:h + 1], "q", h) for h in range(HPC)]
                jobs.append((wkh, wkl, bk, "k", 0))
                jobs.append((wvh, wvl, bv, "v", 0))

                for wh_ap, wl_ap, b_ap, kind, h in jobs:
                    pl = ppsum.tile([P, TS], F32, tag="plin")
                    terms = [(wh_ap, xth), (wh_ap, xtl), (wl_ap, xth)]
                    nmm = len(terms) * (HO // 2)
                    i = 0
                    for w_ap, x_ap in terms:
                        for hp in range(HO // 2):
                            hs = slice(2 * hp, 2 * hp + 2)
                            nc.tensor.matmul(pl[:], w_ap[:, hs, :], x_ap[:, hs, :],
                                             start=(i == 0), stop=(i == nmm - 1),
                                             perf_mode=DR)
                            i += 1
                    if kind in ("q", "k"):
                        # A = (pl + 64 b) * cos/64 ; B = (pl + 64 b) * sin_rot/64
                        # rope = A + rot64(B)
                        ta = rtmp.tile([P, TS], F32, tag="ta")
                        nc.vector.scalar_tensor_tensor(
                            ta[:], pl[:], b_ap, cosT[:, tsl], ALU.add, ALU.mult)
                        tb = rtmp.tile([P, TS], F32, tag="tb")
                        nc.vector.scalar_tensor_tensor(
                            tb[:], pl[:], b_ap, sinTr[:, tsl], ALU.add, ALU.mult)
                        bs = rtmp.tile([P, TS], F32, tag="bs")
                        nc.gpsimd.dma_start(bs[0:64, :], tb[64:128, :])
                        nc.gpsimd.dma_start(bs[64:128, :], tb[0:64, :])
                        dst = qT[:, h, tsl] if kind == "q" else kT[:, tsl]
                        nc.vector.tensor_add(dst, ta[:], bs[:])
                    else:
                        lin = rtmp.tile([P, TS], BF16, tag="lin")
                        nc.vector.tensor_scalar(lin[:], pl[:], 1.0 / WSCL, b_ap,
                                                ALU.mult, ALU.add)
                        for tt in range(TS // P):
                            vp = vpsum.tile([P, P], BF16, tag="vtp")
                            nc.tensor.transpose(vp[:], lin[:, tt * P:(tt + 1) * P],
                                                iden[:])
                            nc.vector.tensor_copy(vN[:, sl * (TS // P) + tt, :], vp[:])

        # ---- Phase B + C: attention (S^T layout flash) + output projection ----
        with ExitStack() as pb:
            # PSUM budget (8 banks): stp 2x[P,1024]=4, avp 2, rsp 1, opp 1
            stp = pb.enter_context(tc.tile_pool(name="stp", bufs=2, space="PSUM"))
            avp = pb.enter_context(tc.tile_pool(name="avp", bufs=2, space="PSUM"))
            rsp = pb.enter_context(tc.tile_pool(name="rsp", bufs=1, space="PSUM"))
            opp = pb.enter_context(tc.tile_pool(name="opp", bufs=1, space="PSUM"))
            ptp = pb.enter_context(tc.tile_pool(name="ptp", bufs=14))
            nstage = pb.enter_context(tc.tile_pool(name="nstage", bufs=3))
            qtmp = pb.enter_context(tc.tile_pool(name="qtmp", bufs=8))
            ostage = pb.enter_context(tc.tile_pool(name="ostage", bufs=2))

            for sl in range(NSL):
                tsl = slice(sl * TS, (sl + 1) * TS)
                n_s = 4 * sl + 4          # causal s tiles for this slice
                ng = n_s // 2
                for h in range(HPC):
                  # high priority: when both are ready the PE prefers
                  # attention; projection work drifts into the exp-latency
                  # stall windows as filler
                  with tc.high_priority(offset=110):
                      av = avp.tile([P, TS], F32, tag="av")
                      rs = rsp.tile([P, TS], F32, tag="rs")
                      pend3 = None
                      rcnt = 0
                      n_rsmm = (sl + 2) // 2   # ceil((sl+1)/2) octet matmuls
                      for g in range(ng):
                          st = stp.tile([P, 2 * TS], F32, tag="st")
                          pt = ptp.tile([P, 2 * TS], BF16, tag="pt")
                          # r >= 0 marks a diagonal-region s tile: its first
                          # r*P t-columns are fully masked, so skip them in the
                          # matmuls and exp, and mask only the diagonal block.
                          offs = [max(2 * g + i - 4 * sl, 0) * P for i in range(2)]
                          for i in range(2):
                              s_tile = 2 * g + i
                              off = offs[i]
                              nc.tensor.matmul(
                                  st[:, i * TS + off:(i + 1) * TS],
                                  kT[:, s_tile * P:(s_tile + 1) * P],
                                  qT[:, h, sl * TS + off:(sl + 1) * TS],
                                  start=True, stop=True)
                          if offs[1] == 0:
                              nc.scalar.activation(pt[:], st[:], AF.Exp, scale=SCALE)
                          else:
                              # one exp spanning both segments (incl. the
                              # stale gap [TS : TS+off1], zeroed right after)
                              # -- saves the 352-cycle ACT overhead of a
                              # second call in the exp-paced inner loop
                              off0, off1 = offs
                              nc.scalar.activation(pt[:, off0:], st[:, off0:],
                                                   AF.Exp, scale=SCALE)
                              if off0:
                                  nc.gpsimd.memset(pt[:, :off0], 0.0)
                              nc.gpsimd.memset(pt[:, TS:TS + off1], 0.0)
                              for i in range(2):
                                  c0 = i * TS + offs[i]
                                  nc.vector.tensor_mul(pt[:, c0:c0 + P],
                                                       pt[:, c0:c0 + P], tri[:])
                          for i in range(2):
                              s_tile = 2 * g + i
                              off = offs[i]
                              seg = pt[:, i * TS + off:(i + 1) * TS]
                              nc.tensor.matmul(av[:, off:TS], vN[:, s_tile, :], seg,
                                               start=(s_tile == 0),
                                               stop=(s_tile == n_s - 1))
                          # Rowsum: diag segments are zero-padded below the
                          # diagonal, so full-width tree adds on the DVE fold 4
                          # segments into one tile -> one PE matmul per octet
                          # instead of one per segment.
                          if g % 2 == 0:
                              pt_even = pt
                          else:
                              qd = g // 2
                              t1 = qtmp.tile([P, TS], BF16, tag="q1")
                              nc.vector.tensor_add(t1[:], pt_even[:, :TS],
                                                   pt_even[:, TS:])
                              t2 = qtmp.tile([P, TS], BF16, tag="q2")
                              nc.vector.tensor_add(t2[:], pt[:, :TS], pt[:, TS:])
                              t3 = qtmp.tile([P, TS], BF16, tag="q3",
                                             name=f"t3{qd % 2}")
                              nc.vector.tensor_add(t3[:], t1[:], t2[:])
                              if pend3 is None and qd < sl:
                                  pend3 = t3      # wait for a partner quad
                              else:
                                  if pend3 is not None:
                                      t4 = qtmp.tile([P, TS], BF16, tag="q4")
                                      nc.vector.tensor_add(t4[:], pend3[:], t3[:])
                                      rhs8 = t4
                                      pend3 = None
                                  else:
                                      rhs8 = t3   # odd leftover quad
                                  nc.tensor.matmul(rs[:], ones[:], rhs8[:],
                                                   start=(rcnt == 0),
                                                   stop=(rcnt == n_rsmm - 1))
                                  rcnt += 1
                      # rec = 8 / rowsum (ones is 1/8); aom = 8 * attn_out
                      rec = nstage.tile([P, TS], F32, tag="rec")
                      nc.vector.reciprocal(rec[:], rs[:])
                      aom = nstage.tile([P, TS], F32, tag="aom")
                      nc.vector.tensor_mul(aom[:], av[:], rec[:])
                      # fp8 hi/lo split of the (scaled) attention output; Pool
                      # engine (SBUF-only reads)
                      nc.gpsimd.tensor_copy(aoh[:, h, tsl], aom[:])
                      nc.gpsimd.tensor_tensor(aol[:, h, tsl], aom[:],
                                              aoh[:, h, tsl], ALU.subtract)

                # output projection for this slice's 4 row blocks: fp8-DR over
                # (d x head) pairs, 3 hi/lo cross terms; PSUM/512 -> fp16
                fin = sl == NSL - 1
                for tt4 in range(4):
                    t_tile = 4 * sl + tt4
                    trow = slice(t_tile * P, (t_tile + 1) * P)
                    ot = ostage.tile([P, HID], F16, tag="ot")
                    for upair in range(2):
                        if fin:
                            op2 = stp.tile([P, 2 * TS], F32, tag="st", name="op2")
                            ops = [op2[:, :TS], op2[:, TS:]]
                        else:
                            ops = [opp.tile([P, TS], F32, tag="op", name=f"op{ui}")[:]
                                   for ui in range(2)]
                        for ui in range(2):
                            u0 = (upair * 2 + ui) * TS
                            usl = slice(u0, u0 + TS)
                            nc.tensor.matmul(ops[ui], aoh[:, :, trow],
                                             woh[:, :, usl],
                                             start=True, stop=False, perf_mode=DR)
                            nc.tensor.matmul(ops[ui], aoh[:, :, trow],
                                             wol[:, :, usl],
                                             start=False, stop=False, perf_mode=DR)
                            nc.tensor.matmul(ops[ui], aol[:, :, trow],
                                             woh[:, :, usl],
                                             start=False, stop=True, perf_mode=DR)
                        for ui in range(2):
                            u0 = (upair * 2 + ui) * TS
                            if ui == 0:
                                nc.vector.tensor_scalar(
                                    ot[:, u0:u0 + TS], ops[ui],
                                    1.0 / (WSCL * AOSCL), None, ALU.mult)
                            else:
                                nc.scalar.activation(
                                    ot[:, u0:u0 + TS], ops[ui], AF.Copy,
                                    scale=1.0 / (WSCL * AOSCL))
                        if fin:
                            # finer store granularity so the very last DMA
                            # (which the kernel-exit drain waits on) is small
                            uhalf = slice(upair * 2 * TS, (upair + 1) * 2 * TS)
                            nc.sync.dma_start(o_dram[trow, uhalf], ot[:, uhalf])
                    if not fin:
                        nc.gpsimd.dma_start(o_dram[trow, :], ot[:])


def _build_nc():
    nc = bacc.Bacc("TRN2", target_bir_lowering=False, debug=False,
                   enable_asserts=False, num_devices=NCORES)
    io = {}

    def din(name, shape, dt):
        io[name] = nc.dram_tensor(name, shape, dt, kind="ExternalInput").ap()

    din("xtth", [NSL, P, HO, TS], F8)        # x^T hi, pre-tiled per slice
    din("xttl", [NSL, P, HO, TS], F8)        # x^T lo
    din("wqth", [P, HO, HPC * D], F8)        # 64*wq hi
    din("wqtl", [P, HO, HPC * D], F8)
    din("wkth", [P, HO, D], F8)
    din("wktl", [P, HO, D], F8)
    din("wvth", [P, HO, D], F8)
    din("wvtl", [P, HO, D], F8)
    din("woth", [P, HPC, HID], F8)
    din("wotl", [P, HPC, HID], F8)
    din("cost", [P, T], BF16)                # cos/64, transposed
    din("sintr", [P, T], BF16)               # signed sin/64, partition-rotated
    din("iden", [P, P], BF16)
    din("ones", [P, P], BF16)                # 1/8
    din("tri", [P, P], BF16)
    din("bq", [P, HPC], F32)                 # 64*bq
    din("bk", [P, 1], F32)                   # 64*bk
    din("bv", [P, 1], F32)                   # bv (applied after /64)
    o = nc.dram_tensor("o_part", [T, HID], F16, kind="ExternalOutput").ap()
    _emit(nc, io, o)
    nc.compile()
    return nc


def _get_nc():
    if "nc" not in _CACHE:
        _CACHE["nc"] = _build_nc()
    return _CACHE["nc"]


def _split8(a):
    hi = a.astype(NPF8)
    lo = (a - hi.astype(np.float32)).astype(NPF8)
    return hi, lo


def _consts():
    if "consts" in _CACHE:
        return _CACHE["consts"]
    iden = np.eye(P, dtype=np.float32)
    onesm = np.full((P, P), 1.0 / AOSCL, np.float32)
    tri = np.triu(np.ones((P, P), np.float32))
    _CACHE["consts"] = tuple(a.astype(NPBF16) for a in (iden, onesm, tri))
    return _CACHE["consts"]


def kernel(x, cos, sin, wq, bq, wk, bk, wv, bv, wo):
    x = np.asarray(x, dtype=np.float32)
    cos = np.asarray(cos, dtype=np.float32)
    sin = np.asarray(sin, dtype=np.float32)
    wq = np.asarray(wq, dtype=np.float32)
    bq = np.asarray(bq, dtype=np.float32)
    wk = np.asarray(wk, dtype=np.float32)
    bk = np.asarray(bk, dtype=np.float32)
    wv = np.asarray(wv, dtype=np.float32)
    bv = np.asarray(bv, dtype=np.float32)
    wo = np.asarray(wo, dtype=np.float32)

    nc = _get_nc()
    iden, onesm, tri = _consts()

    # x^T tiled: xtt[sl, p, ho, c] = x[0, sl*TS + c, ho*P + p]; fp8 hi/lo
    xT = np.ascontiguousarray(x[0].T)                             # [HID, T]
    xtt = np.ascontiguousarray(
        xT.reshape(HO, P, NSL, TS).transpose(2, 1, 0, 3))         # [NSL,P,HO,TS]
    xtth, xttl = _split8(xtt)

    # cos/64 transposed; sin signed (-sin for d<64), /64, partition-rotated
    cosT = np.ascontiguousarray(cos.T / WSCL).astype(NPBF16)      # [P, T]
    sgn = np.where(np.arange(P) < P // 2, -1.0, 1.0)[:, None]
    sinS = (sin.T * sgn) / WSCL                                   # [P, T]
    sinTr = np.ascontiguousarray(np.roll(sinS, P // 2, axis=0)).astype(NPBF16)

    def wtile(wslice):  # [J, HID] -> [P, HO, J] with h = ho*P + p
        J = wslice.shape[0]
        return np.ascontiguousarray(
            wslice.T.reshape(HO, P, J).transpose(1, 0, 2)) * WSCL

    in_maps = []
    for c in range(NCORES):
        j0 = c * HPC * D
        kvh = c // (NCORES // NKV)
        wqth, wqtl = _split8(wtile(wq[j0:j0 + HPC * D]))          # [P, HO, 256]
        wkth, wktl = _split8(wtile(wk[kvh * D:(kvh + 1) * D]))    # [P, HO, 128]
        wvth, wvtl = _split8(wtile(wv[kvh * D:(kvh + 1) * D]))
        # woT: [P, HPC, HID] with j = h*P + p
        wot = np.ascontiguousarray(
            wo[:, j0:j0 + HPC * D].T.reshape(HPC, P, HID)
            .transpose(1, 0, 2)) * WSCL
        woth, wotl = _split8(wot)
        bqt = np.ascontiguousarray(
            bq[j0:j0 + HPC * D].reshape(HPC, P).T) * WSCL
        bkt = bk[kvh * D:(kvh + 1) * D].reshape(P, 1) * WSCL
        bvt = bv[kvh * D:(kvh + 1) * D].reshape(P, 1)
        in_maps.append({
            "xtth": xtth, "xttl": xttl,
            "wqth": wqth, "wqtl": wqtl, "wkth": wkth, "wktl": wktl,
            "wvth": wvth, "wvtl": wvtl, "woth": woth, "wotl": wotl,
            "cost": cosT, "sintr": sinTr, "iden": iden, "ones": onesm,
            "tri": tri,
            "bq": bqt.astype(np.float32), "bk": bkt.astype(np.float32),
            "bv": bvt.astype(np.float32),
        })

    res = run_bass_kernel_spmd(nc, in_maps, list(range(NCORES)))
    out = np.zeros((T, HID), np.float32)
    for c in range(NCORES):
        out += res.results[c]["o_part"].astype(np.float32)
    return out.reshape(1, T, HID)


# revision 17
# speedup vs baseline: 1.1220x; 1.1220x over previous
"""Trainium2 Bass kernel for GQA causal attention (nn_Attention_89816356094768).

Math (per reference):
  q = x @ wq.T + bq ; k = x @ wk.T + bk ; v = x @ wv.T + bv
  RoPE on q, k; S = q @ k.T * D**-0.5 with causal mask; P = softmax(S)
  out = (P @ v) reassembled over heads @ wo.T

Sharding: tensor-parallel over heads across 8 cores. Core c owns q heads
(2c, 2c+1) and kv head c//4. Each core computes its two heads' attention and
a row-parallel partial of the output projection; the host sums the 8 partials
(fp16 partials, summed in fp32).

v2 changes vs the bf16 baseline:
- q/k/v and output projections run as fp8e4m3 DoubleRow matmuls (contraction
  pairs packed on the partition dim, 0.5 cycles/row on half-width output =
  4x bf16 per k-tile). Full precision is kept with a hi+lo split of both
  operands and 3 cross terms (hi*hi + hi*lo + lo*hi), costing 0.75x the bf16
  cycles. Weights are pre-scaled by 64 on the host (w std 0.012 sits below
  e4m3's normal range; the 1/64 is folded into downstream scalars); the
  attention output is scaled by 8 via the rowsum ones-matrix (host-side
  ones/8) before its fp8 split, and the o-projection PSUM is divided by 512
  in the PSUM->fp16 copy.
- RoPE: rotate-half via the PE rot matrix (as v1); the bias is fused into
  the PSUM->SBUF lin copy ((pl/64) + b via tensor_scalar) and into the cos
  product ((pl + 64b) * cos/64 via scalar_tensor_tensor on the Pool engine),
  so no separate bias pass exists.
- Attention (scores/softmax/PV/rowsum) stays bf16: scores contract over only
  d=128 so DoubleRow cannot beat bf16 there, and P would need an on-device
  hi/lo split (2 extra full passes over the score matrix) to keep precision.
- Output partials are fp16 (half the store traffic), one store per t-tile.
Attention units are emitted under tc.high_priority(offset=110) so projection
work drifts into the exp-latency stall windows as PE filler.
"""

import numpy as np
import ml_dtypes
from contextlib import ExitStack

from concourse import bacc, tile, mybir
from concourse.bass_utils import run_bass_kernel_spmd

NQ, NKV, D = 16, 2, 128
HID = 2048
T = 4096
SCALE = D ** -0.5
NCORES = 8
HPC = NQ // NCORES          # q heads per core
P = 128                     # partitions
TS = 512                    # t-slice width (matmul moving free dim)
NT = T // P                 # 32 t tiles
NSL = T // TS               # 8 t slices
HO = HID // P               # 16 hidden k-tiles
BF16 = mybir.dt.bfloat16
F32 = mybir.dt.float32
F16 = mybir.dt.float16
F8 = mybir.dt.float8e4
AF = mybir.ActivationFunctionType
ALU = mybir.AluOpType
DR = mybir.MatmulPerfMode.DoubleRow
NPBF16 = ml_dtypes.bfloat16
NPF8 = ml_dtypes.float8_e4m3

WSCL = 64.0                 # fp8 pre-scale on weights
AOSCL = 8.0                 # fp8 pre-scale on attention output (via ones/8)

_CACHE = {}


def _emit(nc, io, o_dram):
    with ExitStack() as top:
        tc = top.enter_context(tile.TileContext(nc))
        const = top.enter_context(tc.tile_pool(name="const", bufs=1))
        persist = top.enter_context(tc.tile_pool(name="persist", bufs=1))

        def cload(name, shape, dt, eng=None):
            t = const.tile(shape, dt, tag=name)
            (eng or nc.sync).dma_start(t[:], io[name][:])
            return t

        xs_pool = top.enter_context(tc.tile_pool(name="xs", bufs=2))

        # Load order: the first projection term (w_hi * x_hi) only needs the
        # hi parts, so stream those first in chunks, then the lo parts, then
        # phase-B constants.
        wqh = const.tile([P, HO, HPC * D], F8, tag="wqth")
        wql = const.tile([P, HO, HPC * D], F8, tag="wqtl")
        xt0h = xs_pool.tile([P, HO, TS], F8, tag="xth", name="xt0h")
        xt0l = xs_pool.tile([P, HO, TS], F8, tag="xtl", name="xt0l")
        bq = wkh = wvh = wkl = wvl = None
        for ch in range(8):
            hs = slice(2 * ch, 2 * (ch + 1))
            nc.sync.dma_start(wqh[:, hs, :], io["wqth"][:, hs, :])
            nc.sync.dma_start(xt0h[:, hs, :], io["xtth"][0, :, hs, :])
            if ch == 1:
                bq = cload("bq", [P, HPC], F32)
            elif ch == 3:
                wkh = cload("wkth", [P, HO, D], F8)
            elif ch == 5:
                wvh = cload("wvth", [P, HO, D], F8)
            elif ch == 6:
                nc.sync.dma_start(xt0l[:, :8, :], io["xttl"][0, :, :8, :])
            elif ch == 7:
                nc.sync.dma_start(xt0l[:, 8:, :], io["xttl"][0, :, 8:, :])
        bk = cload("bk", [P, 1], F32)
        bv = cload("bv", [P, 1], F32)
        wql = cload("wqtl", [P, HO, HPC * D], F8)
        wkl = cload("wktl", [P, HO, D], F8)
        wvl = cload("wvtl", [P, HO, D], F8)
        rot = cload("rot", [P, P], BF16)
        iden = cload("iden", [P, P], BF16)
        xt1h = xs_pool.tile([P, HO, TS], F8, tag="xth", name="xt1h")
        xt1l = xs_pool.tile([P, HO, TS], F8, tag="xtl", name="xt1l")
        nc.sync.dma_start(xt1h[:], io["xtth"][1])
        nc.sync.dma_start(xt1l[:], io["xttl"][1])
        # RoPE tables stream per t-slice inside phase A
        cosT = const.tile([P, T], BF16, tag="cost")    # cos/64
        sinT = const.tile([P, T], BF16, tag="sint")    # sin (unscaled)
        ones = cload("ones", [P, P], BF16, eng=nc.gpsimd)   # value 1/8
        tri = cload("tri", [P, P], BF16, eng=nc.gpsimd)
        woh = const.tile([P, HPC, HID], F8, tag="woth")  # loaded at sl==2
        wol = const.tile([P, HPC, HID], F8, tag="wotl")

        qT = persist.tile([P, HPC, T], BF16, tag="qT")     # [d, h, t]
        kT = persist.tile([P, T], BF16, tag="kT")          # [d, s]
        vN = persist.tile([P, NT, P], BF16, tag="vN")      # [s_in, s_tile, d]
        aoh = persist.tile([P, HPC, T], F8, tag="aoh")     # [d, h, t] hi
        aol = persist.tile([P, HPC, T], F8, tag="aol")     # [d, h, t] lo

        # ---- Phase A: q/k/v fp8-DR projections, RoPE, v transpose ----
        with ExitStack() as pa:
            ppsum = pa.enter_context(tc.tile_pool(name="ppsum", bufs=4, space="PSUM"))
            rpsum = pa.enter_context(tc.tile_pool(name="rpsum", bufs=2, space="PSUM"))
            vpsum = pa.enter_context(tc.tile_pool(name="vpsum", bufs=2, space="PSUM"))
            rtmp = pa.enter_context(tc.tile_pool(name="rtmp", bufs=4))

            for sl in range(NSL):
                tsl = slice(sl * TS, (sl + 1) * TS)
                if sl == 0:
                    xth, xtl = xt0h, xt0l
                elif sl == 1:
                    xth, xtl = xt1h, xt1l
                else:
                    xth = xs_pool.tile([P, HO, TS], F8, tag="xth")
                    xtl = xs_pool.tile([P, HO, TS], F8, tag="xtl")
                    nc.sync.dma_start(xth[:], io["xtth"][sl])
                    nc.sync.dma_start(xtl[:], io["xttl"][sl])
                nc.sync.dma_start(cosT[:, tsl], io["cost"][:, tsl])
                nc.sync.dma_start(sinT[:, tsl], io["sint"][:, tsl])
                if sl == 2:
                    nc.sync.dma_start(woh[:], io["woth"][:])
                elif sl == 3:
                    nc.sync.dma_start(wol[:], io["wotl"][:])

                # (hi weight AP, lo weight AP, bias, kind, head idx)
                jobs = [(wqh[:, :, h * D:(h + 1) * D],
                         wql[:, :, h * D:(h + 1) * D],
                         bq[:, h:h + 1], "q", h) for h in range(HPC)]
                jobs.append((wkh, wkl, bk, "k", 0))
                jobs.append((wvh, wvl, bv, "v", 0))

                for wh_ap, wl_ap, b_ap, kind, h in jobs:
                    pl = ppsum.tile([P, TS], F32, tag="plin")
                    terms = [(wh_ap, xth), (wh_ap, xtl), (wl_ap, xth)]
                    nmm = len(terms) * (HO // 2)
                    i = 0
                    for w_ap, x_ap in terms:
                        for hp in range(HO // 2):
                            hs = slice(2 * hp, 2 * hp + 2)
                            nc.tensor.matmul(pl[:], w_ap[:, hs, :], x_ap[:, hs, :],
                                             start=(i == 0), stop=(i == nmm - 1),
                                             perf_mode=DR)
                            i += 1
                    if kind in ("q", "k"):
                        # lin = pl/64 + b (bf16, feeds the PE rotate-half and
                        # the Pool cos product); tsin = rot(lin) * sin on DVE.
                        lin = rtmp.tile([P, TS], BF16, tag="lin")
                        nc.vector.tensor_scalar(lin[:], pl[:], 1.0 / WSCL, b_ap,
                                                ALU.mult, ALU.add)
                        tcos = rtmp.tile([P, TS], F32, tag="tcos")
                        nc.gpsimd.tensor_mul(tcos[:], lin[:], cosT[:, tsl])
                        rp = rpsum.tile([P, TS], F32, tag="rp")
                        nc.tensor.matmul(rp[:], rot[:], lin[:], start=True,
                                         stop=True)
                        tsin = rtmp.tile([P, TS], F32, tag="tsin")
                        nc.vector.tensor_mul(tsin[:], rp[:], sinT[:, tsl])
                        dst = qT[:, h, tsl] if kind == "q" else kT[:, tsl]
                        nc.vector.tensor_add(dst, tsin[:], tcos[:])
                    else:
                        lin = rtmp.tile([P, TS], BF16, tag="lin")
                        nc.vector.tensor_scalar(lin[:], pl[:], 1.0 / WSCL, b_ap,
                                                ALU.mult, ALU.add)
                        for tt in range(TS // P):
                            vp = vpsum.tile([P, P], BF16, tag="vtp")
                            nc.tensor.transpose(vp[:], lin[:, tt * P:(tt + 1) * P],
                                                iden[:])
                            nc.vector.tensor_copy(vN[:, sl * (TS // P) + tt, :], vp[:])

        # ---- Phase B + C: attention (S^T layout flash) + output projection ----
        with ExitStack() as pb:
            # PSUM budget (8 banks): stp 2x[P,1024]=4, avp 2, rsp 1, opp 1
            stp = pb.enter_context(tc.tile_pool(name="stp", bufs=2, space="PSUM"))
            avp = pb.enter_context(tc.tile_pool(name="avp", bufs=2, space="PSUM"))
            rsp = pb.enter_context(tc.tile_pool(name="rsp", bufs=1, space="PSUM"))
            opp = pb.enter_context(tc.tile_pool(name="opp", bufs=1, space="PSUM"))
            ptp = pb.enter_context(tc.tile_pool(name="ptp", bufs=14))
            nstage = pb.enter_context(tc.tile_pool(name="nstage", bufs=3))
            qtmp = pb.enter_context(tc.tile_pool(name="qtmp", bufs=8))
            ostage = pb.enter_context(tc.tile_pool(name="ostage", bufs=2))

            for sl in range(NSL):
                tsl = slice(sl * TS, (sl + 1) * TS)
                n_s = 4 * sl + 4          # causal s tiles for this slice
                ng = n_s // 2
                for h in range(HPC):
                  # high priority: when both are ready the PE prefers
                  # attention; projection work drifts into the exp-latency
                  # stall windows as filler
                  with tc.high_priority(offset=110):
                      av = avp.tile([P, TS], F32, tag="av")
                      rs = rsp.tile([P, TS], F32, tag="rs")
                      pend3 = None
                      rcnt = 0
                      n_rsmm = (sl + 2) // 2   # ceil((sl+1)/2) octet matmuls
                      for g in range(ng):
                          st = stp.tile([P, 2 * TS], F32, tag="st")
                          pt = ptp.tile([P, 2 * TS], BF16, tag="pt")
                          # r >= 0 marks a diagonal-region s tile: its first
                          # r*P t-columns are fully masked, so skip them in the
                          # matmuls and exp, and mask only the diagonal block.
                          offs = [max(2 * g + i - 4 * sl, 0) * P for i in range(2)]
                          for i in range(2):
                              s_tile = 2 * g + i
                              off = offs[i]
                              nc.tensor.matmul(
                                  st[:, i * TS + off:(i + 1) * TS],
                                  kT[:, s_tile * P:(s_tile + 1) * P],
                                  qT[:, h, sl * TS + off:(sl + 1) * TS],
                                  start=True, stop=True)
                          if offs[1] == 0:
                              nc.scalar.activation(pt[:], st[:], AF.Exp, scale=SCALE)
                          else:
                              # one exp spanning both segments (incl. the
                              # stale gap [TS : TS+off1], zeroed right after)
                              # -- saves the 352-cycle ACT overhead of a
                              # second call in the exp-paced inner loop
                              off0, off1 = offs
                              nc.scalar.activation(pt[:, off0:], st[:, off0:],
                                                   AF.Exp, scale=SCALE)
                              if off0:
                                  nc.gpsimd.memset(pt[:, :off0], 0.0)
                              nc.gpsimd.memset(pt[:, TS:TS + off1], 0.0)
                              for i in range(2):
                                  c0 = i * TS + offs[i]
                                  nc.vector.tensor_mul(pt[:, c0:c0 + P],
                                                       pt[:, c0:c0 + P], tri[:])
                          for i in range(2):
                              s_tile = 2 * g + i
                              off = offs[i]
                              seg = pt[:, i * TS + off:(i + 1) * TS]
                              nc.tensor.matmul(av[:, off:TS], vN[:, s_tile, :], seg,
                                               start=(s_tile == 0),
                                               stop=(s_tile == n_s - 1))
                          # Rowsum: diag segments are zero-padded below the
                          # diagonal, so full-width tree adds on the DVE fold 4
                          # segments into one tile -> one PE matmul per octet
                          # instead of one per segment.
                          if g % 2 == 0:
                              pt_even = pt
                          else:
                              qd = g // 2
                              t1 = qtmp.tile([P, TS], BF16, tag="q1")
                              nc.vector.tensor_add(t1[:], pt_even[:, :TS],
                                                   pt_even[:, TS:])
                              t2 = qtmp.tile([P, TS], BF16, tag="q2")
                              nc.vector.tensor_add(t2[:], pt[:, :TS], pt[:, TS:])
                              t3 = qtmp.tile([P, TS], BF16, tag="q3",
                                             name=f"t3{qd % 2}")
                              nc.vector.tensor_add(t3[:], t1[:], t2[:])
                              if pend3 is None and qd < sl:
                                  pend3 = t3      # wait for a partner quad
                              else:
                                  if pend3 is not None:
                                      t4 = qtmp.tile([P, TS], BF16, tag="q4")
                                      nc.vector.tensor_add(t4[:], pend3[:], t3[:])
                                      rhs8 = t4
                                      pend3 = None
                                  else:
                                      rhs8 = t3   # odd leftover quad
                                  nc.tensor.matmul(rs[:], ones[:], rhs8[:],
                                                   start=(rcnt == 0),
                                                   stop=(rcnt == n_rsmm - 1))
                                  rcnt += 1
                      # rec = 8 / rowsum (ones is 1/8); aom = 8 * attn_out
                      rec = nstage.tile([P, TS], F32, tag="rec")
                      nc.vector.reciprocal(rec[:], rs[:])
                      aom = nstage.tile([P, TS], F32, tag="aom")
                      nc.vector.tensor_mul(aom[:], av[:], rec[:])
                      # fp8 hi/lo split of the (scaled) attention output; Pool
                      # engine (SBUF-only reads)
                      nc.gpsimd.tensor_copy(aoh[:, h, tsl], aom[:])
                      nc.gpsimd.tensor_tensor(aol[:, h, tsl], aom[:],
                                              aoh[:, h, tsl], ALU.subtract)

                # output projection for this slice's 4 row blocks: fp8-DR over
                # (d x head) pairs, 3 hi/lo cross terms; PSUM/512 -> fp16
                fin = sl == NSL - 1
                for tt4 in range(4):
                    t_tile = 4 * sl + tt4
                    trow = slice(t_tile * P, (t_tile + 1) * P)
                    ot = ostage.tile([P, HID], F16, tag="ot")
                    for upair in range(2):
                        if fin:
                            op2 = stp.tile([P, 2 * TS], F32, tag="st", name="op2")
                            ops = [op2[:, :TS], op2[:, TS:]]
                        else:
                            ops = [opp.tile([P, TS], F32, tag="op", name=f"op{ui}")[:]
                                   for ui in range(2)]
                        for ui in range(2):
                            u0 = (upair * 2 + ui) * TS
                            usl = slice(u0, u0 + TS)
                            nc.tensor.matmul(ops[ui], aoh[:, :, trow],
                                             woh[:, :, usl],
                                             start=True, stop=False, perf_mode=DR)
                            nc.tensor.matmul(ops[ui], aoh[:, :, trow],
                                             wol[:, :, usl],
                                             start=False, stop=False, perf_mode=DR)
                            nc.tensor.matmul(ops[ui], aol[:, :, trow],
                                             woh[:, :, usl],
                                             start=False, stop=True, perf_mode=DR)
                        for ui in range(2):
                            u0 = (upair * 2 + ui) * TS
                            if ui == 0:
                                nc.vector.tensor_scalar(
                                    ot[:, u0:u0 + TS], ops[ui],
                                    1.0 / (WSCL * AOSCL), None, ALU.mult)
                            else:
                                nc.scalar.activation(
                                    ot[:, u0:u0 + TS], ops[ui], AF.Copy,
                                    scale=1.0 / (WSCL * AOSCL))
                        if fin:
                            # finer store granularity so the very last DMA
                            # (which the kernel-exit drain waits on) is small
                            uhalf = slice(upair * 2 * TS, (upair + 1) * 2 * TS)
                            nc.sync.dma_start(o_dram[trow, uhalf], ot[:, uhalf])
                    if not fin:
                        nc.sync.dma_start(o_dram[trow, :], ot[:])


def _build_nc():
    nc = bacc.Bacc("TRN2", target_bir_lowering=False, debug=False,
                   enable_asserts=False, num_devices=NCORES)
    io = {}

    def din(name, shape, dt):
        io[name] = nc.dram_tensor(name, shape, dt, kind="ExternalInput").ap()

    din("xtth", [NSL, P, HO, TS], F8)        # x^T hi, pre-tiled per slice
    din("xttl", [NSL, P, HO, TS], F8)        # x^T lo
    din("wqth", [P, HO, HPC * D], F8)        # 64*wq hi
    din("wqtl", [P, HO, HPC * D], F8)
    din("wkth", [P, HO, D], F8)
    din("wktl", [P, HO, D], F8)
    din("wvth", [P, HO, D], F8)
    din("wvtl", [P, HO, D], F8)
    din("woth", [P, HPC, HID], F8)
    din("wotl", [P, HPC, HID], F8)
    din("cost", [P, T], BF16)                # cos, transposed
    din("sint", [P, T], BF16)                # sin, transposed
    din("rot", [P, P], BF16)
    din("iden", [P, P], BF16)
    din("ones", [P, P], BF16)                # 1/8
    din("tri", [P, P], BF16)
    din("bq", [P, HPC], F32)
    din("bk", [P, 1], F32)
    din("bv", [P, 1], F32)
    o = nc.dram_tensor("o_part", [T, HID], F16, kind="ExternalOutput").ap()
    _emit(nc, io, o)
    nc.compile()
    return nc


def _get_nc():
    if "nc" not in _CACHE:
        _CACHE["nc"] = _build_nc()
    return _CACHE["nc"]


def _split8(a):
    hi = a.astype(NPF8)
    lo = (a - hi.astype(np.float32)).astype(NPF8)
    return hi, lo


def _consts():
    if "consts" in _CACHE:
        return _CACHE["consts"]
    # rotate_half as a matmul on lin: out[d,t] = sum_e R[e,d] lin[e,t]
    R = np.zeros((P, P), np.float32)
    for e in range(P // 2):
        R[e, e + P // 2] = 1.0      # d >= 64 takes +q[d-64]
    for e in range(P // 2, P):
        R[e, e - P // 2] = -1.0     # d < 64 takes -q[d+64]
    iden = np.eye(P, dtype=np.float32)
    onesm = np.full((P, P), 1.0 / AOSCL, np.float32)
    tri = np.triu(np.ones((P, P), np.float32))
    _CACHE["consts"] = tuple(a.astype(NPBF16) for a in (R, iden, onesm, tri))
    return _CACHE["consts"]


def kernel(x, cos, sin, wq, bq, wk, bk, wv, bv, wo):
    x = np.asarray(x, dtype=np.float32)
    cos = np.asarray(cos, dtype=np.float32)
    sin = np.asarray(sin, dtype=np.float32)
    wq = np.asarray(wq, dtype=np.float32)
    bq = np.asarray(bq, dtype=np.float32)
    wk = np.asarray(wk, dtype=np.float32)
    bk = np.asarray(bk, dtype=np.float32)
    wv = np.asarray(wv, dtype=np.float32)
    bv = np.asarray(bv, dtype=np.float32)
    wo = np.asarray(wo, dtype=np.float32)

    nc = _get_nc()
    R, iden, onesm, tri = _consts()

    # x^T tiled: xtt[sl, p, ho, c] = x[0, sl*TS + c, ho*P + p]; fp8 hi/lo
    xT = np.ascontiguousarray(x[0].T)                             # [HID, T]
    xtt = np.ascontiguousarray(
        xT.reshape(HO, P, NSL, TS).transpose(2, 1, 0, 3))         # [NSL,P,HO,TS]
    xtth, xttl = _split8(xtt)

    cosT = np.ascontiguousarray(cos.T).astype(NPBF16)             # [P, T]
    sinT = np.ascontiguousarray(sin.T).astype(NPBF16)

    def wtile(wslice):  # [J, HID] -> [P, HO, J] with h = ho*P + p
        J = wslice.shape[0]
        return np.ascontiguousarray(
            wslice.T.reshape(HO, P, J).transpose(1, 0, 2)) * WSCL

    in_maps = []
    for c in range(NCORES):
        j0 = c * HPC * D
        kvh = c // (NCORES // NKV)
        wqth, wqtl = _split8(wtile(wq[j0:j0 + HPC * D]))          # [P, HO, 256]
        wkth, wktl = _split8(wtile(wk[kvh * D:(kvh + 1) * D]))    # [P, HO, 128]
        wvth, wvtl = _split8(wtile(wv[kvh * D:(kvh + 1) * D]))
        # woT: [P, HPC, HID] with j = h*P + p
        wot = np.ascontiguousarray(
            wo[:, j0:j0 + HPC * D].T.reshape(HPC, P, HID)
            .transpose(1, 0, 2)) * WSCL
        woth, wotl = _split8(wot)
        bqt = np.ascontiguousarray(bq[j0:j0 + HPC * D].reshape(HPC, P).T)
        bkt = bk[kvh * D:(kvh + 1) * D].reshape(P, 1)
        bvt = bv[kvh * D:(kvh + 1) * D].reshape(P, 1)
        in_maps.append({
            "xtth": xtth, "xttl": xttl,
            "wqth": wqth, "wqtl": wqtl, "wkth": wkth, "wktl": wktl,
            "wvth": wvth, "wvtl": wvtl, "woth": woth, "wotl": wotl,
            "cost": cosT, "sint": sinT, "rot": R, "iden": iden, "ones": onesm,
            "tri": tri,
            "bq": bqt.astype(np.float32), "bk": bkt.astype(np.float32),
            "bv": bvt.astype(np.float32),
        })

    res = run_bass_kernel_spmd(nc, in_maps, list(range(NCORES)))
    out = np.zeros((T, HID), np.float32)
    for c in range(NCORES):
        out += res.results[c]["o_part"].astype(np.float32)
    return out.reshape(1, T, HID)
